# revision 1
# baseline (speedup 1.0000x reference)
"""Trainium2 Bass kernel for the KB criterion loss.

Math
----
reference:
    diff[b,i,j] = probs[b,j] - probs[b,i]
    loss = sum_ij mean_b (diff^2 * C[i,j]) / (n_pos + 1e-8),  n_pos = count(C > 0)

Expanding the square removes the [B,N,N] intermediate entirely:

    sum_b (P[b,i] - P[b,j])^2 = S2_i + S2_j - 2*G_ij
        with S2_j = sum_b P[b,j]^2   and   G = P^T P  (Gram matrix)

so   total = sum_ij C_ij * D_ij,   D = S2_i + S2_j - 2 G_ij
     loss  = (total / B) / (n_pos + 1e-8)

Sharding (8 cores)
------------------
Shard C by rows: core k owns rows S_k = [128k, 128k+128). P is replicated.
Each core moves 0.5MB of C + 0.5MB of P (vs 4MB of C with batch-parallel
sharding). Inputs are column-rolled by 128k so every core runs the same
program with its own row block mapped to local columns [0:128).

Per-core pipeline:
  1. DMA P (full, rolled) and C (row block, rolled) to SBUF.
  2. DVE: Psq = P*P.
  3. PE:  S2h = -(1/2) * ones^T @ Psq  (contract over b)  -> [1, N] (2 matmuls)
  4. ACT: copy S2h PSUM->SBUF.
  5. PE:  D' accumulated in PSUM per 512-col bank:
            D' = P_Sk^T @ P  (Gram block, contract over b=128 partitions)
               + S2h_Sk ⊗ 1  (rank-1, K=1)
               + 1 ⊗ S2h     (rank-1, K=1)
          so D' = G - S2_i/2 - S2_j/2 = -D/2.
  6. DVE: tensor_tensor_reduce: (C * D') * (-2), accum per partition -> [128,1].
  7. ACT: Sign(C) with accum_out -> per-partition n_pos counts [128,1].
  8. PE:  ones^T reduction of both columns -> two scalars; DMA [1,2] out.

Host sums the 8 partial pairs (the scalar all-reduce) and finishes the
division.
"""

import numpy as np

import concourse.bass as bass
import concourse.tile as tile
from concourse import mybir
from concourse.bass_utils import run_bass_kernel_spmd

B = 128
N = 1024
NCORES = 8
SH = N // NCORES  # 128 rows of C per core
F32 = mybir.dt.float32
HALF = 512  # fp32 moving-operand max free dim per matmul / PSUM bank


def build_bass() -> bass.Bass:
    nc = bass.Bass()
    p_d = nc.dram_tensor("probs_r", [B, N], F32, kind="ExternalInput")
    c_d = nc.dram_tensor("co_r", [SH, N], F32, kind="ExternalInput")
    o_d = nc.dram_tensor("out", [1, 2], F32, kind="ExternalOutput")

    with tile.TileContext(nc) as tc:
        with (
            tc.tile_pool(name="sb", bufs=1) as sb,
            tc.tile_pool(name="ps", bufs=1, space="PSUM") as ps,
        ):
            p_sb = sb.tile([B, N], F32)
            c_sb = sb.tile([SH, N], F32)
            psq = sb.tile([B, N], F32)
            s2h = sb.tile([1, N], F32)
            ones_row = sb.tile([1, HALF], F32)
            nh_col = sb.tile([B, 1], F32)
            ones_col_act = sb.tile([B, 1], F32)
            pcol = sb.tile([SH, 1], F32)
            npcol = sb.tile([SH, 1], F32)
            scr0 = sb.tile([SH, N], F32)
            scr1 = sb.tile([SH, N], F32)
            scr2 = sb.tile([SH, N], F32)
            out_sb = sb.tile([1, 2], F32)

            d_ps = ps.tile([B, N], F32)  # 2 banks
            s2_ps0 = ps.tile([1, HALF], F32)
            s2_ps1 = ps.tile([1, HALF], F32)
            fin0 = ps.tile([1, 1], F32)
            fin1 = ps.tile([1, 1], F32)

            # trn2 LDWEIGHTS carries ONE sync-wait slot, so each matmul's
            # operands must trace back to a single upstream engine.
            # Constants are therefore born on the engine their consumer
            # pairs with: DVE consts pair with DVE-produced psq; ACT
            # consts pair with ACT-produced s2h / pcol / npcol.
            nc.vector.memset(nh_col, -0.5)
            # ACT-born ones: Copy(x*0 + 1) — input never contributes
            nc.scalar.activation(
                ones_row, ones_row, mybir.ActivationFunctionType.Copy,
                bias=1.0, scale=0.0,
            )
            nc.scalar.activation(
                ones_col_act, ones_col_act, mybir.ActivationFunctionType.Copy,
                bias=1.0, scale=0.0,
            )

            # loads; P first (it heads the S2 critical path). One dma_start
            # per input measured faster than split-queue variants (29.8 vs
            # 31.1 us): queues share the 16 SDMA engines and extra DMA
            # semaphores cost wait-split NOP stalls.
            nc.sync.dma_start(out=p_sb, in_=p_d[:, :])
            nc.sync.dma_start(out=c_sb, in_=c_d[:, :])

            # Psq = P*P
            nc.vector.tensor_mul(psq, p_sb, p_sb)

            # -S2/2 = (-1/2)·colsum_b(Psq) -> [1, N] in PSUM, then to SBUF
            nc.tensor.matmul(s2_ps0, nh_col, psq[:, 0:HALF], start=True, stop=True)
            nc.tensor.matmul(s2_ps1, nh_col, psq[:, HALF:N], start=True, stop=True)
            nc.scalar.copy(s2h[0:1, 0:HALF], s2_ps0)
            nc.scalar.copy(s2h[0:1, HALF:N], s2_ps1)

            # D' = G - S2_i/2 - S2_j/2  (= -D/2) accumulated per PSUM bank.
            # lhsT = p_sb slice (same DMA sem as rhs -> one wait).
            for h in range(2):
                js = slice(HALF * h, HALF * (h + 1))
                nc.tensor.matmul(
                    d_ps[:, js], p_sb[:, 0:SH], p_sb[:, js], start=True, stop=False
                )
                nc.tensor.matmul(
                    d_ps[:, js], s2h[0:1, 0:SH], ones_row[0:1, :],
                    start=False, stop=False,
                )
                nc.tensor.matmul(
                    d_ps[:, js], ones_row[0:1, 0:SH], s2h[0:1, js],
                    start=False, stop=True,
                )

            # scr0 = C * D' on DVE; ACT reduce applies the -2 (scale imm):
            # pcol = sum_j -2*C*D' = sum_j C*D
            nc.vector.tensor_mul(scr0, c_sb, d_ps)
            nc.scalar.activation(
                scr1, scr0, mybir.ActivationFunctionType.Copy,
                scale=-2.0, accum_out=pcol,
            )

            # n_pos per partition: sum_j sign(C)  (C >= 0 always)
            nc.scalar.activation(
                scr2, c_sb, mybir.ActivationFunctionType.Sign, accum_out=npcol
            )

            # partition reduce -> scalars (all-ACT operand pairs)
            nc.tensor.matmul(fin0, ones_col_act, pcol, start=True, stop=True)
            nc.tensor.matmul(fin1, ones_col_act, npcol, start=True, stop=True)
            nc.scalar.copy(out_sb[0:1, 0:1], fin0)
            nc.scalar.copy(out_sb[0:1, 1:2], fin1)

            nc.sync.dma_start(out=o_d[:, :], in_=out_sb)

    _split_multi_waits(nc)
    return nc


def _split_multi_waits(nc: bass.Bass):
    """This walrus build accepts only ONE sync-wait per instruction
    ("Too many sync wait commands"). Tile's kernel-tail drain carries one
    wait per live semaphore; peel the extras onto same-engine NOPs that
    each stall on a single semaphore — semantically identical."""
    for bb in nc.main_func.blocks:
        insts = bb.instructions
        i = 0
        while i < len(insts):
            ins = insts[i]
            si = getattr(ins, "sync_info", None)
            if si is not None and si.on_wait is not None and len(si.on_wait) > 1:
                waits = list(si.on_wait)
                nops = []
                for j, w in enumerate(waits[:-1]):
                    nop = mybir.InstNoOp(
                        name=f"{ins.name}-wsplit{j}",
                        sync_info=mybir.SyncInfo(on_wait=[w], on_update=[]),
                        bass_nofuse=True,
                        engine=ins.engine,
                    )
                    nc.register_instruction(nop, overwrite=True)
                    nops.append(nop)
                si.on_wait = [waits[-1]]
                insts[i:i] = nops
                i += len(nops)
            i += 1


_NC = None


def _get_nc() -> bass.Bass:
    global _NC
    if _NC is None:
        _NC = build_bass()
    return _NC


def make_in_maps(probs: np.ndarray, co_matrix: np.ndarray):
    probs = np.ascontiguousarray(np.asarray(probs, dtype=np.float32))
    co_matrix = np.ascontiguousarray(np.asarray(co_matrix, dtype=np.float32))
    in_maps = []
    for k in range(NCORES):
        shift = -SH * k
        p_r = np.ascontiguousarray(np.roll(probs, shift, axis=1))
        c_r = np.ascontiguousarray(
            np.roll(co_matrix[SH * k : SH * (k + 1), :], shift, axis=1)
        )
        in_maps.append({"probs_r": p_r, "co_r": c_r})
    return in_maps


def finish(outs: np.ndarray) -> np.ndarray:
    """outs: [NCORES, 1, 2] per-core (partial_sum, partial_npos)."""
    total = np.float32(outs[:, 0, 0].astype(np.float64).sum())
    npos = np.float32(outs[:, 0, 1].astype(np.float64).sum())
    loss = (total / np.float32(B)) / (npos + np.float32(1e-8))
    return np.array(loss, dtype=np.float32)


def kernel(probs: np.ndarray, co_matrix: np.ndarray) -> np.ndarray:
    nc = _get_nc()
    in_maps = make_in_maps(probs, co_matrix)
    res = run_bass_kernel_spmd(nc, in_maps, list(range(NCORES)))
    outs = np.stack([r["out"] for r in res.results])
    return finish(outs)



# revision 4
# speedup vs baseline: 1.7239x; 1.7239x over previous
"""Trainium2 Bass kernel for the KB criterion loss.

Math
----
reference:
    diff[b,i,j] = probs[b,j] - probs[b,i]
    loss = sum_ij mean_b (diff^2 * C[i,j]) / (n_pos + 1e-8),  n_pos = count(C > 0)

Expanding the square removes the [B,N,N] intermediate:

    total = sum_i S2_i r_i + sum_j S2_j c_j - 2 sum_b P_b^T C P_b
        with S2_j = sum_b P[b,j]^2, r_i = sum_j C_ij, c_j = sum_i C_ij
    loss  = (total / B) / (n_pos + 1e-8)

Sharding (8 cores)
------------------
Shard C by rows: core k owns rows S_k = [128k, 128k+128). P replicated.
Inputs are shipped TRANSPOSED (j on partitions) and column-rolled by 128k
so every core runs the same program; the contraction over j=1024 runs as
8 accumulating chunks of K=128.

Host packs one bf16 input tile per core, pk[p, c, 0:257]:
    pk[p,c,0:128] = P[b, gj]^T   (gj = (128c + p + 128k) % 1024; col b)
    pk[p,c,128]   = 1.0          (ones column -> row sums r_i)
    pk[p,c,129:257] = C[S_k, gj]^T  (col i local)

Per-core pipeline (bf16 matmuls, fp32 PSUM):
  PE  mm1 (8x): M[i, 0:129]  += ct_c^T @ [pt|1]_c   -> M[i,b] = (C P^T)[i,b], M[i,128] = r_i
  DVE psq = pt^2   (one 2x-mode multiply)
  PE  mm2 (8x): M2[i, 0:128] += ct_c^T @ psq_c      -> sum_ib M2 = sum_j S2_j c_j
  ACT sign(ct) accum -> npos per partition; copy r from PSUM to SBUF
  DVE pm    = sum_b pt_0 * M        (tensor_tensor_reduce)   -> col 2
      part1 = sum_b psq_0 * r       (tensor_scalar, 4x mode) -> col 1
      t2    = sum_b M2              (tensor_reduce)          -> col 0
  DMA out cols [128, 4] = [t2, part1, pm, npos]

Host sums the 8x[128,4] partials (the scalar all-reduce) and finishes:
loss = (sum(t2 + part1 - 2 pm) / B) / (sum(npos) + 1e-8).
"""

import ml_dtypes
import numpy as np

import concourse.bass as bass
import concourse.tile as tile
from concourse import mybir
from concourse.bass_utils import run_bass_kernel_spmd

B = 128
N = 1024
NCORES = 8
SH = N // NCORES  # 128 rows of C per core
NCH = N // 128  # 8 contraction chunks
F32 = mybir.dt.float32
BF16 = mybir.dt.bfloat16


def build_bass() -> bass.Bass:
    nc = bass.Bass()
    pk_d = nc.dram_tensor("pk", [128, NCH, 257], BF16, kind="ExternalInput")
    o_d = nc.dram_tensor("out", [128, 4], F32, kind="ExternalOutput")

    with tile.TileContext(nc) as tc:
        with (
            tc.tile_pool(name="sb", bufs=1) as sb,
            tc.tile_pool(name="ps", bufs=1, space="PSUM") as ps,
        ):
            pk = sb.tile([128, NCH, 257], BF16)
            psq = sb.tile([128, NCH, 128], BF16)
            sgn = sb.tile([128, NCH, 128], BF16)
            scr_a = sb.tile([128, 128], BF16)
            scr_b = sb.tile([128, 128], BF16)
            r_sb = sb.tile([128, 1], F32)
            cols = sb.tile([128, 4], F32)

            m_ps = ps.tile([128, 129], F32)
            m2_ps = ps.tile([128, 128], F32)

            nc.sync.dma_start(out=pk, in_=pk_d[:, :, :])

            # PE group 1: M[i, 0:128] = (C P^T)[i, b], M[i, 128] = r_i
            for c in range(NCH):
                nc.tensor.matmul(
                    m_ps,
                    pk[:, c, 129:257],
                    pk[:, c, 0:129],
                    start=(c == 0),
                    stop=(c == NCH - 1),
                )

            # DVE: psq = pt^2 (2x mode: bf16 packed, SBUF)
            nc.vector.tensor_mul(psq, pk[:, :, 0:128], pk[:, :, 0:128])

            # PE group 2: M2[i, 0:128] = sum_j ct[j,i] * psq[j,b]
            for c in range(NCH):
                nc.tensor.matmul(
                    m2_ps,
                    pk[:, c, 129:257],
                    psq[:, c, :],
                    start=(c == 0),
                    stop=(c == NCH - 1),
                )

            # ACT: npos counts, then PSUM->SBUF copy of r
            nc.scalar.activation(
                sgn,
                pk[:, :, 129:257],
                mybir.ActivationFunctionType.Sign,
                accum_out=cols[:, 3:4],
            )
            nc.scalar.copy(r_sb, m_ps[:, 128:129])

            # DVE tail
            nc.vector.scalar_tensor_tensor(
                out=scr_b,
                in0=pk[:, 0, 0:128],
                scalar=1.0,
                in1=m_ps[:, 0:128],
                op0=mybir.AluOpType.mult,
                op1=mybir.AluOpType.mult,
                accum_out=cols[:, 2:3],
            )
            nc.vector.tensor_scalar(
                scr_a,
                psq[:, 0, :],
                r_sb,
                None,
                mybir.AluOpType.mult,
                op1=mybir.AluOpType.add,
                accum_out=cols[:, 1:2],
            )
            nc.vector.tensor_reduce(
                out=cols[:, 0:1],
                in_=m2_ps,
                axis=mybir.AxisListType.X,
                op=mybir.AluOpType.add,
            )

            nc.sync.dma_start(out=o_d[:, :], in_=cols)

    _split_multi_waits(nc)
    return nc


def _split_multi_waits(nc: bass.Bass):
    """This walrus build accepts only ONE sync-wait per instruction
    ("Too many sync wait commands"). Peel extras onto same-engine NOPs that
    each stall on a single semaphore — semantically identical."""
    for bb in nc.main_func.blocks:
        insts = bb.instructions
        i = 0
        while i < len(insts):
            ins = insts[i]
            si = getattr(ins, "sync_info", None)
            if si is not None and si.on_wait is not None and len(si.on_wait) > 1:
                waits = list(si.on_wait)
                nops = []
                for j, w in enumerate(waits[:-1]):
                    nop = mybir.InstNoOp(
                        name=f"{ins.name}-wsplit{j}",
                        sync_info=mybir.SyncInfo(on_wait=[w], on_update=[]),
                        bass_nofuse=True,
                        engine=ins.engine,
                    )
                    nc.register_instruction(nop, overwrite=True)
                    nops.append(nop)
                si.on_wait = [waits[-1]]
                insts[i:i] = nops
                i += len(nops)
            i += 1


_NC = None


def _get_nc() -> bass.Bass:
    global _NC
    if _NC is None:
        _NC = build_bass()
    return _NC


def make_in_maps(probs: np.ndarray, co_matrix: np.ndarray):
    P = np.ascontiguousarray(np.asarray(probs, dtype=np.float32))
    C = np.ascontiguousarray(np.asarray(co_matrix, dtype=np.float32))
    PT = P.T  # [N(j), B(b)]
    in_maps = []
    for k in range(NCORES):
        sh = SH * k
        ptr = np.roll(PT, -sh, axis=0).reshape(NCH, 128, B).transpose(1, 0, 2)
        ctr = (
            np.roll(C[sh : sh + SH, :].T, -sh, axis=0)
            .reshape(NCH, 128, SH)
            .transpose(1, 0, 2)
        )
        buf = np.empty((128, NCH, 257), dtype=ml_dtypes.bfloat16)
        buf[:, :, 0:128] = ptr
        buf[:, :, 128] = 1.0
        buf[:, :, 129:257] = ctr
        in_maps.append({"pk": buf})
    return in_maps


def finish(outs: np.ndarray) -> np.ndarray:
    """outs: [NCORES, 128, 4] per-core columns (t2, part1, pm, npos)."""
    o = outs.astype(np.float64)
    total = o[:, :, 0].sum() + o[:, :, 1].sum() - 2.0 * o[:, :, 2].sum()
    npos = o[:, :, 3].sum()
    loss = (total / float(B)) / (npos + 1e-8)
    return np.array(loss, dtype=np.float32)


def kernel(probs: np.ndarray, co_matrix: np.ndarray) -> np.ndarray:
    nc = _get_nc()
    in_maps = make_in_maps(probs, co_matrix)
    res = run_bass_kernel_spmd(nc, in_maps, list(range(NCORES)))
    outs = np.stack([r["out"] for r in res.results])
    return finish(outs)
